# revision 7
# baseline (speedup 1.0000x reference)
"""Banded-KNN local-distances kernel for Trainium2 (8 NeuronCores).

Problem: x[4, 8192, 3] -> sorted distances to the +-16 index-neighbors per
point, invalid slots zeroed, output [4, 1, 8192*32].

Sharding: core c handles (batch b = c//2, half h = c%2): 4096 query rows.
Each core receives a halo slice x[b, n0-16 : n0+4096+16] padded with a huge
sentinel point (1e15) outside the global range; sentinel distances (~1.7e15)
sort to the tail naturally and are zeroed by a per-core mask after the sort.

On-device per core:
  - one blob load C[128, 1216]: [:, :1024] = post-sort zero-mask,
    [:, 1024:] = fat halo tile (partition p holds halo rows 32p..32p+63)
  - banded d^2 for all 32 offsets via broadcast-subtract + ACT square
    + 2 strided adds per 16-offset block -> D[128, 1024] laid out (row r, slot j)
  - 15-stage bitonic sort network along the free axis (32-wide rows), ping-pong
    DVE min/max with strided/reversed access patterns
  - mask multiply (zeros sentinel tail slots on edge partitions), ACT sqrt
  - contiguous DMA out (SBUF layout == row-major DRAM layout)
"""

import dataclasses
import sys

import numpy as np

if "/opt/trn_rl_repo" not in sys.path:
    sys.path.insert(0, "/opt/trn_rl_repo")

import concourse.bass as bass
import concourse.bacc as bacc_mod
import concourse.mybir as mybir
from concourse.tile import TileContext
from concourse.bass_utils import run_bass_kernel_spmd

B = 4
N = 8192
K = 16
H = N // 2          # rows per core
P = 128             # partitions
R = H // P          # 32 rows per partition
W = 2 * K           # 32 band slots per row
FD = R * W          # 1024 free elements per partition
FAT = 192           # 64 halo rows * 3 floats per partition
CW = FD + FAT       # blob width
PAD = 1.0e15        # sentinel coordinate for out-of-range halo rows
F32 = mybir.dt.float32


def _v(tile_ap, off, pairs):
    """Strided view of a tile at free-element offset `off` with explicit
    [step, count] free-dim pairs (keeps the partition dim pair)."""
    a = tile_ap[:, off : off + 1]
    return dataclasses.replace(a, ap=[list(a.ap[0])] + [list(p) for p in pairs])


def _bitonic_stages():
    stages = []
    for s in [2, 4, 8, 16, 32]:
        stages.append(("flip", s))
        j = s // 4
        while j >= 1:
            stages.append(("straight", j))
            j //= 2
    return stages


def _build_nc():
    nc = bacc_mod.Bacc()
    blob = nc.dram_tensor("blob", [P, CW], F32, kind="ExternalInput")
    out = nc.dram_tensor("out", [P, FD], F32, kind="ExternalOutput")

    mn = mybir.AluOpType.min
    mx = mybir.AluOpType.max

    with TileContext(nc) as tc:
        with tc.tile_pool(name="pool", bufs=1) as pool:
            C = pool.tile([P, CW], F32)
            tmpL = pool.tile([P, 1536], F32)
            tmpR = pool.tile([P, 1536], F32)
            sqL = pool.tile([P, 1536], F32)
            sqR = pool.tile([P, 1536], F32)
            s1L = pool.tile([P, 512], F32)
            s1R = pool.tile([P, 512], F32)
            D = pool.tile([P, FD], F32)
            E = pool.tile([P, FD], F32)
            O = pool.tile([P, FD], F32)

            # fat halo first (gates compute), split across 2 DMA queues;
            # mask load overlaps the whole compute+sort phase on a 3rd queue
            nc.sync.dma_start(C[:, FD : FD + 96], blob[:, FD : FD + 96])
            nc.sync.dma_start(C[:, FD + 96 : CW], blob[:, FD + 96 : CW])
            nc.sync.dma_start(C[:, 0:FD], blob[:, 0:FD])

            # banded diff: blocks (left offsets -16..-1 -> slots 0..15 at
            # fat-base 0; right offsets +1..+16 -> slots 16..31 at base 51)
            center = C[:, FD + 48 : FD + 144]
            center_b = center.unsqueeze(1).broadcast_to((P, 16, 96))
            for base, tmp, sq, s1, j0 in (
                (FD + 0, tmpL, sqL, s1L, 0),
                (FD + 51, tmpR, sqR, s1R, 16),
            ):
                shifted = _v(C, base, [[3, 16], [1, 96]])
                dst3 = _v(tmp, 0, [[96, 16], [1, 96]])
                nc.vector.tensor_sub(dst3, shifted, center_b)
                nc.scalar.activation(
                    sq[:, :], tmp[:, :], mybir.ActivationFunctionType.Square
                )
                # reduce over the 3 channels; (oi, r) iteration everywhere
                c0 = _v(sq, 0, [[96, 16], [3, 32]])
                c1 = _v(sq, 1, [[96, 16], [3, 32]])
                c2 = _v(sq, 2, [[96, 16], [3, 32]])
                s1v = _v(s1, 0, [[32, 16], [1, 32]])
                nc.vector.tensor_add(s1v, c0, c1)
                # transposed write into D: element (oi, r) -> r*32 + j0 + oi
                dD = _v(D, j0, [[1, 16], [32, 32]])
                nc.vector.tensor_add(dD, s1v, c2)

            # bitonic sort of each 32-wide row along the free axis
            bufs = [D, E]
            for k, (kind, s) in enumerate(_bitonic_stages()):
                sb, db = bufs[k % 2], bufs[(k + 1) % 2]
                if kind == "flip":
                    nb, hs = FD // s, s // 2
                    a_lo = _v(sb, 0, [[s, nb], [1, hs]])
                    b_rev = _v(sb, s - 1, [[s, nb], [-1, hs]])
                    d_lo = _v(db, 0, [[s, nb], [1, hs]])
                    nc.vector.tensor_tensor(d_lo, a_lo, b_rev, mn)
                    b_hi = _v(sb, hs, [[s, nb], [1, hs]])
                    a_rev = _v(sb, hs - 1, [[s, nb], [-1, hs]])
                    d_hi = _v(db, hs, [[s, nb], [1, hs]])
                    nc.vector.tensor_tensor(d_hi, b_hi, a_rev, mx)
                else:
                    j = s
                    nb = FD // (2 * j)
                    a_ = _v(sb, 0, [[2 * j, nb], [1, j]])
                    b_ = _v(sb, j, [[2 * j, nb], [1, j]])
                    d_lo = _v(db, 0, [[2 * j, nb], [1, j]])
                    d_hi = _v(db, j, [[2 * j, nb], [1, j]])
                    nc.vector.tensor_tensor(d_lo, a_, b_, mn)
                    nc.vector.tensor_tensor(d_hi, a_, b_, mx)
            final = bufs[len(_bitonic_stages()) % 2]
            other = bufs[(len(_bitonic_stages()) + 1) % 2]

            # zero the sentinel tail slots, then sqrt; processed in two
            # column halves so half-0's stores overlap half-1's compute,
            # with stores split across DMA queues
            for g0 in (0, FD // 2):
                cs = slice(g0, g0 + FD // 2)
                nc.vector.tensor_mul(other[:, cs], final[:, cs], C[:, cs])
                nc.scalar.activation(
                    O[:, cs], other[:, cs], mybir.ActivationFunctionType.Sqrt
                )
                for q0 in (g0, g0 + FD // 4):
                    qs = slice(q0, q0 + FD // 4)
                    nc.sync.dma_start(out[:, qs], O[:, qs])
    nc.finalize()
    return nc


_NC_CACHE = {}


def _get_nc():
    if "nc" not in _NC_CACHE:
        _NC_CACHE["nc"] = _build_nc()
    return _NC_CACHE["nc"]


def _make_mask(c):
    m = np.ones((P, FD), np.float32)
    if c % 2 == 0:
        # global rows 0..15 live in partition 0, rows r=0..15; row r has
        # 16-r invalid slots -> sorted tail positions r*32+16+r .. r*32+31
        for r in range(16):
            m[0, r * 32 + 16 + r : r * 32 + 32] = 0.0
    else:
        # global rows 8176..8191 live in partition 127, rows r=16..31;
        # row r has r-15 invalid slots -> tail positions r*32+47-r .. r*32+31
        for r in range(16, 32):
            m[127, r * 32 + 47 - r : r * 32 + 32] = 0.0
    return m


def _make_in_maps(x):
    in_maps = []
    for c in range(8):
        b, h = c // 2, c % 2
        n0 = h * H
        xh = np.full((H + 2 * K, 3), PAD, np.float32)
        lo, hi = n0 - K, n0 + H + K
        clo, chi = max(lo, 0), min(hi, N)
        xh[clo - lo : chi - lo] = x[b, clo:chi]
        flat = xh.reshape(-1)
        # fat tile: partition p holds halo elements [96p : 96p+192]
        idx = (96 * np.arange(P))[:, None] + np.arange(FAT)[None, :]
        fat = flat[idx]
        blob = np.concatenate([_make_mask(c), fat], axis=1)
        in_maps.append({"blob": np.ascontiguousarray(blob, np.float32)})
    return in_maps


def _run(x, trace=False, **kwargs):
    nc = _get_nc()
    in_maps = _make_in_maps(x)
    res = run_bass_kernel_spmd(
        nc, in_maps, core_ids=list(range(8)), trace=trace, **kwargs
    )
    out = np.zeros((B, N * W), np.float32)
    for c in range(8):
        b, h = c // 2, c % 2
        out[b, h * H * W : (h + 1) * H * W] = np.asarray(
            res.results[c]["out"], np.float32
        ).reshape(-1)
    return out.reshape(B, 1, N * W), res


def kernel(x):
    x = np.asarray(x, np.float32)
    out, _ = _run(x)
    return out
